# revision 1
# baseline (speedup 1.0000x reference)
"""CosfacePairwiseLoss Trainium2 kernel (8 NeuronCores, Bass/Tile).

Strategy:
- Loss is invariant under joint row/col permutation of the pairwise matrix, so
  the host sorts rows by label; each row's positives then live in a narrow
  diagonal band (max group size ~25 for the 1000-label/8192-row regime).
- Rows are sharded 1024/core. Each core normalizes its rows (bf16), AllGathers
  the normalized features, and computes its [1024, 8192] similarity block with
  bf16 matmuls (f32 PSUM).
- Dense logsumexp(neg): host supplies a 0/1 positives mask (bf16); one fused
  DVE op computes 30*sim - 1000*mask from PSUM, ACT exp accumulates row sums.
  Positives underflow to exactly 0, matching the reference's -10-fill behavior
  in f32.
- logsumexp(pos): a small [128, 256] band matmul per row-tile (dynamic rhs
  offset = 1024*core + 128*m) + the same host mask slice gives
  sum(exp(9 - 30*sim)) over the group.
- Per-row losses go back to the host, which averages (the only unsharded op).
"""
import os
import numpy as np
import ml_dtypes

import concourse.bass as bass
import concourse.bacc as bacc
import concourse.mybir as mybir
import concourse.tile as tile
from concourse.bass_utils import run_bass_kernel_spmd

F32 = mybir.dt.float32
BF16 = mybir.dt.bfloat16
AF = mybir.ActivationFunctionType
ALU = mybir.AluOpType

N, D, NCORES = 8192, 512, 8
R = N // NCORES  # rows per core
MT = R // 128  # row-tiles per core
NCH = N // 512  # 512-wide column chunks
W = 256  # band window width
PAD = 64  # fT padding each side
NP_ = N + 2 * PAD  # padded columns

_CACHED = {}


def _build_nc(sim_variant: bool = False, skip_epi: bool = False, split_tpose: int = 1):
    n_dev = 1 if sim_variant else NCORES
    nc = bacc.Bacc("TRN2", target_bir_lowering=False, debug=False, num_devices=n_dev)

    feat_in = nc.dram_tensor("feat_in", [R, D], F32, kind="ExternalInput").ap()
    mask_in = nc.dram_tensor("mask_in", [R, NP_], BF16, kind="ExternalInput").ap()
    o_loss = nc.dram_tensor("o_loss", [128, MT], F32, kind="ExternalOutput").ap()

    with tile.TileContext(nc) as tc:
        with (
            tc.tile_pool(name="io", bufs=3) as io,
            tc.tile_pool(name="fbp", bufs=3) as fbp,
            tc.tile_pool(name="stats", bufs=8) as stats,
            tc.tile_pool(name="singles", bufs=1) as singles,
            tc.tile_pool(name="ftp", bufs=1) as ftp,
            tc.tile_pool(name="maskp", bufs=2) as maskp,
            tc.tile_pool(name="up", bufs=3) as upool,
            tc.tile_pool(name="ep", bufs=3) as epool,
            tc.tile_pool(name="bsmall", bufs=2) as bsmall,
            tc.tile_pool(name="nsp", bufs=2) as nsp,
            tc.tile_pool(name="psmain", bufs=6, space="PSUM") as psmain,
            tc.tile_pool(name="psband", bufs=2, space="PSUM") as psband,
            tc.tile_pool(name="dram", bufs=1, space="DRAM") as dram,
        ):
            cc_in = dram.tile([R, D], BF16)
            cc_out = dram.tile([N, D], BF16, addr_space="Shared")

            bias150 = singles.tile([128, 1], F32)
            nc.vector.memset(bias150, -150.0)
            losses = singles.tile([128, MT], F32)

            # ---- Phase A: normalize own rows -> bf16, stage to DRAM ----
            for m in range(MT):
                x = io.tile([128, D], F32, tag="x")
                nc.sync.dma_start(out=x, in_=feat_in[bass.ts(m, 128), :])
                scr = io.tile([128, D], F32, tag="scr")
                ss = stats.tile([128, 1], F32, tag="ss")
                nc.scalar.activation(scr, x, AF.Square, accum_out=ss)
                ssc = stats.tile([128, 1], F32, tag="ssc")
                nc.vector.tensor_scalar_max(ssc, ss, 1e-16)
                lnss = stats.tile([128, 1], F32, tag="lnss")
                nc.scalar.activation(lnss, ssc, AF.Ln)
                rinv = stats.tile([128, 1], F32, tag="rinv")
                nc.scalar.activation(rinv, lnss, AF.Exp, scale=-0.5)
                fb = fbp.tile([128, D], BF16, tag="fb")
                nc.vector.tensor_scalar_mul(fb, x, rinv)
                nc.sync.dma_start(out=cc_in[bass.ts(m, 128), :], in_=fb)

            # ---- own-block fT (static lhsT source) ----
            ft_own = [singles.tile([128, R], BF16, name=f"ft_own{k}") for k in range(4)]
            for k in range(4):
                nc.sync.dma_start_transpose(
                    out=ft_own[k], in_=cc_in[:, bass.ts(k, 128)]
                )

            # ---- AllGather normalized features ----
            if sim_variant:
                nc.sync.dma_start(out=cc_out[0:R, :], in_=cc_in[:, :])
            else:
                nc.gpsimd.collective_compute(
                    "AllGather",
                    ALU.bypass,
                    replica_groups=[list(range(NCORES))],
                    ins=[cc_in.opt()],
                    outs=[cc_out.opt()],
                )

            # ---- fT_all (padded) ----
            ftall = [singles.tile([128, NP_], BF16, name=f"ftall{k}") for k in range(4)]
            for k in range(4):
                nc.vector.memset(ftall[k][:, 0:PAD], 0.0)
                nc.vector.memset(ftall[k][:, NP_ - PAD : NP_], 0.0)
                piece = N // split_tpose
                for t in range(split_tpose):
                    nc.sync.dma_start_transpose(
                        out=ftall[k][:, PAD + t * piece : PAD + (t + 1) * piece],
                        in_=cc_out[t * piece : (t + 1) * piece, bass.ts(k, 128)],
                    )

            pid_pe = nc.tensor.partition_id()
            pid_dve = nc.vector.partition_id()

            # ---- Phase C: per row-tile ----
            CHUNK_GROUPS = [list(range(0, 6)), list(range(6, 12)), list(range(12, 16))]
            for m in range(MT):
                mt_t = maskp.tile([128, NP_], BF16, tag="mask")
                nc.sync.dma_start(out=mt_t, in_=mask_in[bass.ts(m, 128), :])
                nsum = nsp.tile([128, NCH], F32, tag="nsum")

                psums = {}
                for grp in CHUNK_GROUPS:
                    for k in range(4):
                        for n in grp:
                            if k == 0:
                                psums[n] = psmain.tile([128, 512], F32, tag="ps", name=f"ps{n}")
                            nc.tensor.matmul(
                                psums[n],
                                ft_own[k][:, bass.ts(m, 128)],
                                ftall[k][:, PAD + 512 * n : PAD + 512 * (n + 1)],
                                start=(k == 0),
                                stop=(k == 3),
                            )
                    for n in grp:
                        if skip_epi:
                            u = upool.tile([128, 512], F32, tag="u")
                            nc.vector.tensor_copy(u[:, 0:8], psums[n][:, 0:8])
                            continue
                        u = upool.tile([128, 512], F32, tag="u")
                        nc.vector.scalar_tensor_tensor(
                            u,
                            in0=mt_t[:, PAD + 512 * n : PAD + 512 * (n + 1)],
                            scalar=-33.333333,
                            in1=psums[n],
                            op0=ALU.mult,
                            op1=ALU.add,
                        )
                        e = epool.tile([128, 512], F32, tag="e")
                        nc.scalar.activation(
                            e, u, AF.Exp, scale=30.0, accum_out=nsum[:, n : n + 1]
                        )

                # band (positives) pass
                if skip_epi:
                    nc.vector.memset(losses[:, m : m + 1], 0.0)
                    continue
                off_pe = pid_pe * 1024 + 128 * m
                off_dve = pid_dve * 1024 + 128 * m
                bp = psband.tile([128, W], F32, tag="bps")
                for k in range(4):
                    nc.tensor.matmul(
                        bp,
                        ft_own[k][:, bass.ts(m, 128)],
                        ftall[k][:, bass.ds(off_pe, W)],
                        start=(k == 0),
                        stop=(k == 3),
                    )
                ub = bsmall.tile([128, W], F32, tag="ub")
                nc.vector.scalar_tensor_tensor(
                    ub,
                    in0=mt_t[:, bass.ds(off_dve, W)],
                    scalar=5.3,
                    in1=bp,
                    op0=ALU.mult,
                    op1=ALU.subtract,
                )
                eb = bsmall.tile([128, W], F32, tag="eb")
                pcol = stats.tile([128, 1], F32, tag="pcol")
                nc.scalar.activation(
                    eb, ub, AF.Exp, scale=30.0, bias=bias150, accum_out=pcol
                )

                # combine: loss = softplus(ln(P) + ln(N))
                ncol = stats.tile([128, 1], F32, tag="ncol")
                nc.vector.reduce_sum(ncol, nsum, axis=mybir.AxisListType.X)
                lp = stats.tile([128, 1], F32, tag="lp")
                nc.scalar.activation(lp, pcol, AF.Ln)
                lnn = stats.tile([128, 1], F32, tag="lnn")
                nc.scalar.activation(lnn, ncol, AF.Ln)
                xr = stats.tile([128, 1], F32, tag="xr")
                nc.vector.tensor_tensor(xr, lp, lnn, op=ALU.add)
                er = stats.tile([128, 1], F32, tag="er")
                nc.scalar.activation(er, xr, AF.Exp)
                er1 = stats.tile([128, 1], F32, tag="er1")
                nc.vector.tensor_scalar_add(er1, er, 1.0)
                nc.scalar.activation(losses[:, m : m + 1], er1, AF.Ln)

            nc.sync.dma_start(out=o_loss, in_=losses)

    nc.compile()
    return nc


def kernel(feat: np.ndarray, label: np.ndarray) -> np.ndarray:
    feat = np.asarray(feat, dtype=np.float32)
    label = np.asarray(label)
    assert feat.shape == (N, D) and label.shape == (N,)

    # sort rows by label (loss is permutation invariant)
    perm = np.argsort(label, kind="stable")
    lab_s = np.asarray(label)[perm]
    feat_s = feat[perm]

    # group bounds per row
    lab64 = lab_s.astype(np.int64)
    starts = np.searchsorted(lab64, lab64, side="left")
    ends = np.searchsorted(lab64, lab64, side="right")

    # verify every row's group fits its tile's band window
    rows = np.arange(N)
    tile_of = rows // 128
    woff = tile_of * 128 - PAD  # window [woff, woff + W)
    assert (starts >= woff).all() and (ends <= woff + W).all(), (
        "label group exceeds band window; widen W"
    )

    in_maps = []
    for r in range(NCORES):
        sl = slice(r * R, (r + 1) * R)
        rl = lab64[sl][:, None]
        mask_rows = (rl == lab64[None, :]).astype(ml_dtypes.bfloat16)
        maskp = np.zeros((R, NP_), dtype=ml_dtypes.bfloat16)
        maskp[:, PAD : PAD + N] = mask_rows
        in_maps.append({"feat_in": feat_s[sl], "mask_in": maskp})

    if "nc" not in _CACHED:
        _CACHED["nc"] = _build_nc()
    nc = _CACHED["nc"]

    res = run_bass_kernel_spmd(nc, in_maps, core_ids=list(range(NCORES)))
    loss_rows = np.concatenate(
        [res.results[r]["o_loss"].T.reshape(-1) for r in range(NCORES)]
    )
    return np.float32(loss_rows.mean())



# revision 2
# speedup vs baseline: 2.1326x; 2.1326x over previous
"""CosfacePairwiseLoss Trainium2 kernel (4 NeuronCores, Bass/Tile).

Strategy (v2):
- The amortized per-exec cost through the axon/PJRT dispatch path grows with
  device count (~1.0 ms for 1 core, ~1.2 ms for 4, ~2.0 ms for 8) and device
  compute adds on top of it, so the sweet spot is 4 cores: dispatch stays
  cheap while the GEMM (the only large compute term) still shards 4 ways.
- No collective: every core receives the FULL feat (replicated), normalizes
  all 8192 rows locally (~60 us, cheaper than an AllGather round-trip), and
  builds the full transposed feature matrix fT. Rows are sharded 2048/core
  for the similarity/logsumexp phase only.
- Loss is invariant under joint row/col permutation, so the host sorts rows
  by label; positives then live in a 256-wide diagonal band. Host supplies a
  0/1 positives mask (bf16, padded). Dense pass: one fused DVE op computes
  sim - 33.3*mask from PSUM, ACT exp(30*x) accumulates row sums (positives
  underflow to exactly 0). Band pass handles the positives' logsumexp.
- Each core reduces its row losses to a [128,1] column; the host sums
  4*128 floats and divides by N (the only unsharded work).
"""
import numpy as np
import ml_dtypes

import concourse.bass as bass
import concourse.bacc as bacc
import concourse.mybir as mybir
import concourse.tile as tile
from concourse.bass_utils import run_bass_kernel_spmd

F32 = mybir.dt.float32
BF16 = mybir.dt.bfloat16
AF = mybir.ActivationFunctionType
ALU = mybir.AluOpType

N, D, NCORES = 8192, 512, 4
R = N // NCORES  # rows per core (2048)
MT = R // 128  # row-tiles per core (16)
TT = N // 128  # total row-tiles (64, normalize loop)
NCH = N // 512  # 512-wide column chunks (16)
W = 256  # band window width
PAD = 64  # fT padding each side
NP_ = N + 2 * PAD  # padded columns

_CACHED = {}


def _build_nc():
    nc = bacc.Bacc("TRN2", target_bir_lowering=False, debug=False, num_devices=NCORES)

    feat_in = nc.dram_tensor("feat_in", [N, D], F32, kind="ExternalInput").ap()
    mask_in = nc.dram_tensor("mask_in", [R, NP_], BF16, kind="ExternalInput").ap()
    o_loss = nc.dram_tensor("o_loss", [128, 1], F32, kind="ExternalOutput").ap()

    with tile.TileContext(nc) as tc:
        with (
            tc.tile_pool(name="io", bufs=3) as io,
            tc.tile_pool(name="fbp", bufs=3) as fbp,
            tc.tile_pool(name="stats", bufs=8) as stats,
            tc.tile_pool(name="singles", bufs=1) as singles,
            tc.tile_pool(name="maskp", bufs=2) as maskp,
            tc.tile_pool(name="up", bufs=3) as upool,
            tc.tile_pool(name="ep", bufs=3) as epool,
            tc.tile_pool(name="bsmall", bufs=2) as bsmall,
            tc.tile_pool(name="nsp", bufs=2) as nsp,
            tc.tile_pool(name="psmain", bufs=6, space="PSUM") as psmain,
            tc.tile_pool(name="psband", bufs=2, space="PSUM") as psband,
            tc.tile_pool(name="dram", bufs=1, space="DRAM") as dram,
        ):
            cc = dram.tile([N, D], BF16)  # normalized rows, core-local

            bias150 = singles.tile([128, 1], F32)
            nc.vector.memset(bias150, -150.0)
            losses = singles.tile([128, MT], F32)

            # ---- Phase A: normalize ALL rows -> bf16, stage to DRAM ----
            for m in range(TT):
                x = io.tile([128, D], F32, tag="x")
                nc.sync.dma_start(out=x, in_=feat_in[bass.ts(m, 128), :])
                scr = io.tile([128, D], F32, tag="scr")
                ss = stats.tile([128, 1], F32, tag="ss")
                nc.scalar.activation(scr, x, AF.Square, accum_out=ss)
                ssc = stats.tile([128, 1], F32, tag="ssc")
                nc.vector.tensor_scalar_max(ssc, ss, 1e-16)
                lnss = stats.tile([128, 1], F32, tag="lnss")
                nc.scalar.activation(lnss, ssc, AF.Ln)
                rinv = stats.tile([128, 1], F32, tag="rinv")
                nc.scalar.activation(rinv, lnss, AF.Exp, scale=-0.5)
                fb = fbp.tile([128, D], BF16, tag="fb")
                nc.vector.tensor_scalar_mul(fb, x, rinv)
                nc.sync.dma_start(out=cc[bass.ts(m, 128), :], in_=fb)

            # ---- Phase B: full fT (padded) + own-rows slice ----
            ftall = [singles.tile([128, NP_], BF16, name=f"ftall{k}") for k in range(4)]
            for k in range(4):
                nc.vector.memset(ftall[k][:, 0:PAD], 0.0)
                nc.vector.memset(ftall[k][:, NP_ - PAD : NP_], 0.0)
                nc.sync.dma_start_transpose(
                    out=ftall[k][:, PAD : PAD + N], in_=cc[:, bass.ts(k, 128)]
                )

            pid_pe = nc.tensor.partition_id()
            pid_dve = nc.vector.partition_id()

            # own-rows fT slice at a static address (ldweights needs static
            # offsets, so copy out of ftall at the pid-dependent offset once)
            ft_own = [singles.tile([128, R], BF16, name=f"ft_own{k}") for k in range(4)]
            for k in range(4):
                nc.vector.tensor_copy(
                    ft_own[k], ftall[k][:, bass.ds(PAD + pid_dve * R, R)]
                )

            # ---- Phase C: per row-tile ----
            CHUNK_GROUPS = [list(range(0, 6)), list(range(6, 12)), list(range(12, 16))]
            for m in range(MT):
                mt_t = maskp.tile([128, NP_], BF16, tag="mask")
                nc.sync.dma_start(out=mt_t, in_=mask_in[bass.ts(m, 128), :])
                nsum = nsp.tile([128, NCH], F32, tag="nsum")

                psums = {}
                for grp in CHUNK_GROUPS:
                    for k in range(4):
                        for n in grp:
                            if k == 0:
                                psums[n] = psmain.tile(
                                    [128, 512], F32, tag="ps", name=f"ps{n}"
                                )
                            nc.tensor.matmul(
                                psums[n],
                                ft_own[k][:, bass.ts(m, 128)],
                                ftall[k][:, PAD + 512 * n : PAD + 512 * (n + 1)],
                                start=(k == 0),
                                stop=(k == 3),
                            )
                    for n in grp:
                        u = upool.tile([128, 512], F32, tag="u")
                        nc.vector.scalar_tensor_tensor(
                            u,
                            in0=mt_t[:, PAD + 512 * n : PAD + 512 * (n + 1)],
                            scalar=-33.333333,
                            in1=psums[n],
                            op0=ALU.mult,
                            op1=ALU.add,
                        )
                        e = epool.tile([128, 512], F32, tag="e")
                        nc.scalar.activation(
                            e, u, AF.Exp, scale=30.0, accum_out=nsum[:, n : n + 1]
                        )

                # band (positives) pass; window starts at padded col 128*g,
                # g = MT*pid + m the global row-tile index
                off_pe = pid_pe * R + 128 * m
                off_dve = pid_dve * R + 128 * m
                bp = psband.tile([128, W], F32, tag="bps")
                for k in range(4):
                    nc.tensor.matmul(
                        bp,
                        ft_own[k][:, bass.ts(m, 128)],
                        ftall[k][:, bass.ds(off_pe, W)],
                        start=(k == 0),
                        stop=(k == 3),
                    )
                ub = bsmall.tile([128, W], F32, tag="ub")
                nc.vector.scalar_tensor_tensor(
                    ub,
                    in0=mt_t[:, bass.ds(off_dve, W)],
                    scalar=5.3,
                    in1=bp,
                    op0=ALU.mult,
                    op1=ALU.subtract,
                )
                eb = bsmall.tile([128, W], F32, tag="eb")
                pcol = stats.tile([128, 1], F32, tag="pcol")
                nc.scalar.activation(
                    eb, ub, AF.Exp, scale=30.0, bias=bias150, accum_out=pcol
                )

                # combine: loss = softplus(ln(P) + ln(N))
                ncol = stats.tile([128, 1], F32, tag="ncol")
                nc.vector.reduce_sum(ncol, nsum, axis=mybir.AxisListType.X)
                lp = stats.tile([128, 1], F32, tag="lp")
                nc.scalar.activation(lp, pcol, AF.Ln)
                lnn = stats.tile([128, 1], F32, tag="lnn")
                nc.scalar.activation(lnn, ncol, AF.Ln)
                xr = stats.tile([128, 1], F32, tag="xr")
                nc.vector.tensor_tensor(xr, lp, lnn, op=ALU.add)
                er = stats.tile([128, 1], F32, tag="er")
                nc.scalar.activation(er, xr, AF.Exp)
                er1 = stats.tile([128, 1], F32, tag="er1")
                nc.vector.tensor_scalar_add(er1, er, 1.0)
                nc.scalar.activation(losses[:, m : m + 1], er1, AF.Ln)

            lsum = singles.tile([128, 1], F32)
            nc.vector.reduce_sum(lsum, losses, axis=mybir.AxisListType.X)
            nc.sync.dma_start(out=o_loss, in_=lsum)

    nc.compile()
    return nc


def _prep_inputs(feat: np.ndarray, label: np.ndarray):
    """Sort rows by label, build per-core padded band masks."""
    perm = np.argsort(label, kind="stable")
    lab64 = np.asarray(label)[perm].astype(np.int64)
    feat_s = np.ascontiguousarray(np.asarray(feat, dtype=np.float32)[perm])

    # verify every row's group fits its tile's band window
    starts = np.searchsorted(lab64, lab64, side="left")
    ends = np.searchsorted(lab64, lab64, side="right")
    rows = np.arange(N)
    woff = (rows // 128) * 128 - PAD  # window [woff, woff + W)
    assert (starts >= woff).all() and (ends <= woff + W).all(), (
        "label group exceeds band window; widen W"
    )

    in_maps = []
    for c in range(NCORES):
        sl = slice(c * R, (c + 1) * R)
        maskp = np.zeros((R, NP_), dtype=ml_dtypes.bfloat16)
        maskp[:, PAD : PAD + N] = lab64[sl][:, None] == lab64[None, :]
        in_maps.append({"feat_in": feat_s, "mask_in": maskp})
    return in_maps


def kernel(feat: np.ndarray, label: np.ndarray) -> np.ndarray:
    feat = np.asarray(feat, dtype=np.float32)
    label = np.asarray(label)
    assert feat.shape == (N, D) and label.shape == (N,)

    in_maps = _prep_inputs(feat, label)

    if "nc" not in _CACHED:
        _CACHED["nc"] = _build_nc()
    nc = _CACHED["nc"]

    res = run_bass_kernel_spmd(nc, in_maps, core_ids=list(range(NCORES)))
    total = sum(float(res.results[c]["o_loss"].sum()) for c in range(NCORES))
    return np.float32(total / N)


# revision 5
# speedup vs baseline: 2.3576x; 1.1055x over previous
"""CosfacePairwiseLoss Trainium2 kernel (4 NeuronCores, Bass/Tile).

Strategy (v2):
- The amortized per-exec cost through the axon/PJRT dispatch path grows with
  device count (~1.0 ms for 1 core, ~1.2 ms for 4, ~2.0 ms for 8) and device
  compute adds on top of it, so the sweet spot is 4 cores: dispatch stays
  cheap while the GEMM (the only large compute term) still shards 4 ways.
- No collective: every core receives the FULL feat (replicated), normalizes
  all 8192 rows locally (~60 us, cheaper than an AllGather round-trip), and
  builds the full transposed feature matrix fT. Rows are sharded 2048/core
  for the similarity/logsumexp phase only.
- Loss is invariant under joint row/col permutation, so the host sorts rows
  by label; positives then live in a 256-wide diagonal band. Host supplies a
  0/1 positives mask (bf16, padded). Dense pass: one fused DVE op computes
  sim - 33.3*mask from PSUM, ACT exp(30*x) accumulates row sums (positives
  underflow to exactly 0). Band pass handles the positives' logsumexp.
- Each core reduces its row losses to a [128,1] column; the host sums
  4*128 floats and divides by N (the only unsharded work).
"""
import numpy as np
import ml_dtypes

import concourse.bass as bass
import concourse.bacc as bacc
import concourse.mybir as mybir
import concourse.tile as tile
from concourse.bass_utils import run_bass_kernel_spmd

F32 = mybir.dt.float32
BF16 = mybir.dt.bfloat16
F8 = mybir.dt.float8e4
AF = mybir.ActivationFunctionType
ALU = mybir.AluOpType
DR = mybir.MatmulPerfMode.DoubleRow

FP8 = True  # fp8e4m3 double-row dense GEMM (band pass stays bf16)
F8SCALE = 16.0  # features scaled x16 before fp8 quantization; sim comes out x256

N, D, NCORES = 8192, 512, 4
R = N // NCORES  # rows per core (2048)
MT = R // 128  # row-tiles per core (16)
TT = N // 128  # total row-tiles (64, normalize loop)
NCH = N // 512  # 512-wide column chunks (16)
W = 256  # band window width
PAD = 64  # fT padding each side
NP_ = N + 2 * PAD  # padded columns

_CACHED = {}


def _build_nc():
    nc = bacc.Bacc("TRN2", target_bir_lowering=False, debug=False, num_devices=NCORES)

    feat_in = nc.dram_tensor("feat_in", [N, D], F32, kind="ExternalInput").ap()
    mask_in = nc.dram_tensor("mask_in", [R, NP_], BF16, kind="ExternalInput").ap()
    o_loss = nc.dram_tensor("o_loss", [128, 1], F32, kind="ExternalOutput").ap()

    with tile.TileContext(nc) as tc:
        with (
            tc.tile_pool(name="io", bufs=3) as io,
            tc.tile_pool(name="fbp", bufs=3) as fbp,
            tc.tile_pool(name="stats", bufs=8) as stats,
            tc.tile_pool(name="singles", bufs=1) as singles,
            tc.tile_pool(name="maskp", bufs=2) as maskp,
            tc.tile_pool(name="up", bufs=3) as upool,
            tc.tile_pool(name="ep", bufs=3) as epool,
            tc.tile_pool(name="bsmall", bufs=2) as bsmall,
            tc.tile_pool(name="nsp", bufs=2) as nsp,
            tc.tile_pool(name="psmain", bufs=6, space="PSUM") as psmain,
            tc.tile_pool(name="psband", bufs=2, space="PSUM") as psband,
            tc.tile_pool(name="dram", bufs=1, space="DRAM") as dram,
        ):
            cc = dram.tile([N, D], BF16)  # normalized rows, core-local

            bias150 = singles.tile([128, 1], F32)
            nc.vector.memset(bias150, -150.0)
            losses = singles.tile([128, MT], F32)

            # ---- Phase A: normalize ALL rows -> bf16, stage to DRAM ----
            for m in range(TT):
                x = io.tile([128, D], F32, tag="x")
                nc.sync.dma_start(out=x, in_=feat_in[bass.ts(m, 128), :])
                scr = io.tile([128, D], F32, tag="scr")
                ss = stats.tile([128, 1], F32, tag="ss")
                nc.scalar.activation(scr, x, AF.Square, accum_out=ss)
                ssc = stats.tile([128, 1], F32, tag="ssc")
                nc.vector.tensor_scalar_max(ssc, ss, 1e-16)
                lnss = stats.tile([128, 1], F32, tag="lnss")
                nc.scalar.activation(lnss, ssc, AF.Ln)
                rinv = stats.tile([128, 1], F32, tag="rinv")
                nc.scalar.activation(rinv, lnss, AF.Exp, scale=-0.5)
                fb = fbp.tile([128, D], BF16, tag="fb")
                nc.vector.tensor_scalar_mul(fb, x, rinv)
                nc.sync.dma_start(out=cc[bass.ts(m, 128), :], in_=fb)

            # ---- Phase B: full fT (padded) + own-rows slice ----
            ftall = [singles.tile([128, NP_], BF16, name=f"ftall{k}") for k in range(4)]
            for k in range(4):
                nc.vector.memset(ftall[k][:, 0:PAD], 0.0)
                nc.vector.memset(ftall[k][:, NP_ - PAD : NP_], 0.0)
                nc.sync.dma_start_transpose(
                    out=ftall[k][:, PAD : PAD + N], in_=cc[:, bass.ts(k, 128)]
                )

            pid_pe = nc.tensor.partition_id()
            pid_dve = nc.vector.partition_id()

            # own-rows fT slice at a static address (ldweights needs static
            # offsets, so copy out of ftall at the pid-dependent offset once)
            ft_own = [singles.tile([128, R], BF16, name=f"ft_own{k}") for k in range(4)]
            for k in range(4):
                nc.vector.tensor_copy(
                    ft_own[k], ftall[k][:, bass.ds(PAD + pid_dve * R, R)]
                )

            if FP8:
                # scaled fp8 copies, [128, 4 k-planes, cols] for double-row
                ft8 = singles.tile([128, 4, N], F8, name="ft8")
                ft8_own = singles.tile([128, 4, R], F8, name="ft8_own")
                for k in range(4):
                    nc.vector.tensor_scalar_mul(
                        ft8[:, k, :], ftall[k][:, PAD : PAD + N], F8SCALE
                    )
                    nc.vector.tensor_scalar_mul(ft8_own[:, k, :], ft_own[k], F8SCALE)

            # ---- Phase C: per row-tile ----
            CHUNK_GROUPS = [list(range(0, 6)), list(range(6, 12)), list(range(12, 16))]
            for m in range(MT):
                mt_t = maskp.tile([128, NP_], BF16, tag="mask")
                nc.sync.dma_start(out=mt_t, in_=mask_in[bass.ts(m, 128), :])
                nsum = nsp.tile([128, NCH], F32, tag="nsum")

                # psum holds sim (bf16 path) or 256*sim (fp8 path); the STT
                # mask scalar and exp scale compensate so exp(30*sim - 1000*mask)
                # comes out identical either way
                sim_mul = F8SCALE * F8SCALE if FP8 else 1.0
                psums = {}
                for grp in CHUNK_GROUPS:
                    if FP8:
                        for ks in range(2):
                            for n in grp:
                                if ks == 0:
                                    psums[n] = psmain.tile(
                                        [128, 512], F32, tag="ps", name=f"ps{n}"
                                    )
                                nc.tensor.matmul(
                                    psums[n],
                                    ft8_own[:, 2 * ks : 2 * ks + 2, bass.ts(m, 128)],
                                    ft8[:, 2 * ks : 2 * ks + 2, 512 * n : 512 * (n + 1)],
                                    start=(ks == 0),
                                    stop=(ks == 1),
                                    perf_mode=DR,
                                )
                    else:
                        for k in range(4):
                            for n in grp:
                                if k == 0:
                                    psums[n] = psmain.tile(
                                        [128, 512], F32, tag="ps", name=f"ps{n}"
                                    )
                                nc.tensor.matmul(
                                    psums[n],
                                    ft_own[k][:, bass.ts(m, 128)],
                                    ftall[k][:, PAD + 512 * n : PAD + 512 * (n + 1)],
                                    start=(k == 0),
                                    stop=(k == 3),
                                )
                    for n in grp:
                        u = upool.tile([128, 512], F32, tag="u")
                        nc.vector.scalar_tensor_tensor(
                            u,
                            in0=mt_t[:, PAD + 512 * n : PAD + 512 * (n + 1)],
                            scalar=-33.333333 * sim_mul,
                            in1=psums[n],
                            op0=ALU.mult,
                            op1=ALU.add,
                        )
                        e = epool.tile([128, 512], F32, tag="e")
                        nc.scalar.activation(
                            e, u, AF.Exp, scale=30.0 / sim_mul,
                            accum_out=nsum[:, n : n + 1],
                        )

                # band (positives) pass; window starts at padded col 128*g,
                # g = MT*pid + m the global row-tile index
                off_pe = pid_pe * R + 128 * m
                off_dve = pid_dve * R + 128 * m
                bp = psband.tile([128, W], F32, tag="bps")
                for k in range(4):
                    nc.tensor.matmul(
                        bp,
                        ft_own[k][:, bass.ts(m, 128)],
                        ftall[k][:, bass.ds(off_pe, W)],
                        start=(k == 0),
                        stop=(k == 3),
                    )
                ub = bsmall.tile([128, W], F32, tag="ub")
                nc.vector.scalar_tensor_tensor(
                    ub,
                    in0=mt_t[:, bass.ds(off_dve, W)],
                    scalar=5.3,
                    in1=bp,
                    op0=ALU.mult,
                    op1=ALU.subtract,
                )
                eb = bsmall.tile([128, W], F32, tag="eb")
                pcol = stats.tile([128, 1], F32, tag="pcol")
                nc.scalar.activation(
                    eb, ub, AF.Exp, scale=30.0, bias=bias150, accum_out=pcol
                )

                # combine: loss = softplus(ln(P) + ln(N))
                ncol = stats.tile([128, 1], F32, tag="ncol")
                nc.vector.reduce_sum(ncol, nsum, axis=mybir.AxisListType.X)
                lp = stats.tile([128, 1], F32, tag="lp")
                nc.scalar.activation(lp, pcol, AF.Ln)
                lnn = stats.tile([128, 1], F32, tag="lnn")
                nc.scalar.activation(lnn, ncol, AF.Ln)
                xr = stats.tile([128, 1], F32, tag="xr")
                nc.vector.tensor_tensor(xr, lp, lnn, op=ALU.add)
                er = stats.tile([128, 1], F32, tag="er")
                nc.scalar.activation(er, xr, AF.Exp)
                er1 = stats.tile([128, 1], F32, tag="er1")
                nc.vector.tensor_scalar_add(er1, er, 1.0)
                nc.scalar.activation(losses[:, m : m + 1], er1, AF.Ln)

            lsum = singles.tile([128, 1], F32)
            nc.vector.reduce_sum(lsum, losses, axis=mybir.AxisListType.X)
            nc.sync.dma_start(out=o_loss, in_=lsum)

    nc.compile()
    return nc


def _prep_inputs(feat: np.ndarray, label: np.ndarray):
    """Sort rows by label, build per-core padded band masks."""
    perm = np.argsort(label, kind="stable")
    lab64 = np.asarray(label)[perm].astype(np.int64)
    feat_s = np.ascontiguousarray(np.asarray(feat, dtype=np.float32)[perm])

    # verify every row's group fits its tile's band window
    starts = np.searchsorted(lab64, lab64, side="left")
    ends = np.searchsorted(lab64, lab64, side="right")
    rows = np.arange(N)
    woff = (rows // 128) * 128 - PAD  # window [woff, woff + W)
    assert (starts >= woff).all() and (ends <= woff + W).all(), (
        "label group exceeds band window; widen W"
    )

    in_maps = []
    for c in range(NCORES):
        sl = slice(c * R, (c + 1) * R)
        maskp = np.zeros((R, NP_), dtype=ml_dtypes.bfloat16)
        maskp[:, PAD : PAD + N] = lab64[sl][:, None] == lab64[None, :]
        in_maps.append({"feat_in": feat_s, "mask_in": maskp})
    return in_maps


def kernel(feat: np.ndarray, label: np.ndarray) -> np.ndarray:
    feat = np.asarray(feat, dtype=np.float32)
    label = np.asarray(label)
    assert feat.shape == (N, D) and label.shape == (N,)

    in_maps = _prep_inputs(feat, label)

    if "nc" not in _CACHED:
        _CACHED["nc"] = _build_nc()
    nc = _CACHED["nc"]

    res = run_bass_kernel_spmd(nc, in_maps, core_ids=list(range(NCORES)))
    total = sum(float(res.results[c]["o_loss"].sum()) for c in range(NCORES))
    return np.float32(total / N)


# revision 8
# speedup vs baseline: 2.5351x; 1.0753x over previous
"""CosfacePairwiseLoss Trainium2 kernel (4 NeuronCores, Bass/Tile).

Strategy (v3):
- The amortized per-exec cost through the axon/PJRT dispatch path grows with
  device count (~1.0 ms for 1 core, ~1.2 ms for 4, ~2.0 ms for 8) and device
  compute adds on top of it, so the sweet spot is 4 cores: dispatch stays
  cheap while the GEMM + exp (the only large compute terms) still shard 4x.
- No collective: every core receives the FULL feat (replicated, bf16),
  normalizes all 8192 rows locally, transposes, and quantizes to fp8e4m3
  (x16 scale). Rows are sharded 2048/core for the similarity phase.
- Dense pass (fp8 double-row matmuls, 256-contraction per instruction):
  PSUM holds 256*sim. The host-sorted rows put positives in a 256-wide
  diagonal band; a 0/1 mask (fp8) times -8533.33 is added on the POOL engine
  (scalar_tensor_tensor) so positives underflow to exactly 0 in the ACT
  exp(30/256 * x) row-sum accumulation. Chunks are processed in [128,1024]
  PSUM groups so one Pool/ACT instruction covers two 512-chunks.
- Band pass (positives): small fp8 matmul at a dynamic window offset + the
  same mask gives sum(exp(9 - 30*sim)) per row (margin folded into consts).
- Each core reduces its row losses to a [128,1] column; the host sums
  4*128 floats and divides by N (the only unsharded work).
"""
import numpy as np
import ml_dtypes

import concourse.bass as bass
import concourse.bacc as bacc
import concourse.mybir as mybir
import concourse.tile as tile
from concourse.bass_utils import run_bass_kernel_spmd

F32 = mybir.dt.float32
BF16 = mybir.dt.bfloat16
F8 = mybir.dt.float8e4
AF = mybir.ActivationFunctionType
ALU = mybir.AluOpType
DR = mybir.MatmulPerfMode.DoubleRow

F8SCALE = 16.0  # features scaled x16 before fp8 quantization; sim = psum/256
SIMMUL = F8SCALE * F8SCALE

N, D, NCORES = 8192, 512, 4
R = N // NCORES  # rows per core (2048)
MT = R // 128  # row-tiles per core (16)
TT = N // 128  # total row-tiles (64, normalize loop)
W = 256  # band window width
PAD = 64  # fT padding each side
NP_ = N + 2 * PAD  # padded columns
GW = 1024  # psum group width (2 chunks)
NG = N // GW  # psum groups (8)

_CACHED = {}


def _build_nc():
    nc = bacc.Bacc("TRN2", target_bir_lowering=False, debug=False, num_devices=NCORES)

    feat_in = nc.dram_tensor("feat_in", [N, D], BF16, kind="ExternalInput").ap()
    mask_in = nc.dram_tensor("mask_in", [R, NP_], F8, kind="ExternalInput").ap()
    o_loss = nc.dram_tensor("o_loss", [128, 1], F32, kind="ExternalOutput").ap()

    with tile.TileContext(nc) as tc:
        with (
            tc.tile_pool(name="io", bufs=3) as io,
            tc.tile_pool(name="fbp", bufs=3) as fbp,
            tc.tile_pool(name="stats", bufs=8) as stats,
            tc.tile_pool(name="singles", bufs=1) as singles,
            tc.tile_pool(name="ftmp", bufs=2) as ftp,
            tc.tile_pool(name="maskp", bufs=2) as maskp,
            tc.tile_pool(name="up", bufs=3) as upool,
            tc.tile_pool(name="ep", bufs=3) as epool,
            tc.tile_pool(name="bsmall", bufs=2) as bsmall,
            tc.tile_pool(name="nsp", bufs=2) as nsp,
            tc.tile_pool(name="psmain", bufs=3, space="PSUM") as psmain,
            tc.tile_pool(name="psband", bufs=2, space="PSUM") as psband,
            tc.tile_pool(name="dram", bufs=1, space="DRAM") as dram,
        ):
            cc = dram.tile([N, D], BF16)  # normalized rows, core-local

            bias150 = singles.tile([128, 1], F32)
            nc.vector.memset(bias150, -150.0)
            losses = singles.tile([128, MT], F32)

            # ---- Phase A: normalize ALL rows -> bf16, stage to DRAM ----
            for m in range(TT):
                x = io.tile([128, D], BF16, tag="x")
                nc.sync.dma_start(out=x, in_=feat_in[bass.ts(m, 128), :])
                scr = io.tile([128, D], F32, tag="scr")
                ss = stats.tile([128, 1], F32, tag="ss")
                nc.scalar.activation(scr, x, AF.Square, accum_out=ss)
                ssc = stats.tile([128, 1], F32, tag="ssc")
                nc.vector.tensor_scalar_max(ssc, ss, 1e-16)
                lnss = stats.tile([128, 1], F32, tag="lnss")
                nc.scalar.activation(lnss, ssc, AF.Ln)
                rinv = stats.tile([128, 1], F32, tag="rinv")
                nc.scalar.activation(rinv, lnss, AF.Exp, scale=-0.5)
                fb = fbp.tile([128, D], BF16, tag="fb")
                nc.vector.tensor_scalar_mul(fb, x, rinv)
                nc.sync.dma_start(out=cc[bass.ts(m, 128), :], in_=fb)

            pid_pe = nc.tensor.partition_id()
            pid_dve = nc.vector.partition_id()

            # ---- Phase B: transpose each k-slice, quantize to fp8 (x16) ----
            ft8 = singles.tile([128, 4, NP_], F8, name="ft8")
            ft8_own = singles.tile([128, 4, R], F8, name="ft8_own")
            for k in range(4):
                nc.vector.memset(ft8[:, k, 0:PAD], 0.0)
                nc.vector.memset(ft8[:, k, NP_ - PAD : NP_], 0.0)
                ftmp = ftp.tile([128, N], BF16, tag="ftmp")
                nc.sync.dma_start_transpose(out=ftmp, in_=cc[:, bass.ts(k, 128)])
                nc.vector.tensor_scalar_mul(ft8[:, k, PAD : PAD + N], ftmp, F8SCALE)
                nc.vector.tensor_scalar_mul(
                    ft8_own[:, k, :], ftmp[:, bass.ds(pid_dve * R, R)], F8SCALE
                )

            # ---- Phase C: per row-tile ----
            for m in range(MT):
                mt_t = maskp.tile([128, NP_], F8, tag="mask")
                nc.sync.dma_start(out=mt_t, in_=mask_in[bass.ts(m, 128), :])
                nsum = nsp.tile([128, NG], F32, tag="nsum")

                for g in range(NG):
                    ps = psmain.tile([128, GW], F32, tag="ps")
                    for ks in range(2):
                        for half in range(2):
                            nc.tensor.matmul(
                                ps[:, bass.ts(half, 512)],
                                ft8_own[:, 2 * ks : 2 * ks + 2, bass.ts(m, 128)],
                                ft8[
                                    :,
                                    2 * ks : 2 * ks + 2,
                                    PAD + GW * g + 512 * half : PAD
                                    + GW * g
                                    + 512 * (half + 1),
                                ],
                                start=(ks == 0),
                                stop=(ks == 1),
                                perf_mode=DR,
                            )
                    u = upool.tile([128, GW], F32, tag="u")
                    nc.vector.scalar_tensor_tensor(
                        u,
                        in0=mt_t[:, PAD + GW * g : PAD + GW * (g + 1)],
                        scalar=-33.333333 * SIMMUL,
                        in1=ps,
                        op0=ALU.mult,
                        op1=ALU.add,
                    )
                    e = epool.tile([128, GW], BF16, tag="e")
                    nc.scalar.activation(
                        e, u, AF.Exp, scale=30.0 / SIMMUL,
                        accum_out=nsum[:, g : g + 1],
                    )

                # band (positives) pass; window starts at padded col 128*gt,
                # gt = MT*pid + m the global row-tile index
                off_pe = pid_pe * R + 128 * m
                off_dve = pid_dve * R + 128 * m
                bp = psband.tile([128, W], F32, tag="bps")
                for k in range(4):
                    nc.tensor.matmul(
                        bp,
                        ft8_own[:, k, bass.ts(m, 128)],
                        ft8[:, k, bass.ds(off_pe, W)],
                        start=(k == 0),
                        stop=(k == 3),
                    )
                ub = bsmall.tile([128, W], F32, tag="ub")
                nc.vector.scalar_tensor_tensor(
                    ub,
                    in0=mt_t[:, bass.ds(off_dve, W)],
                    scalar=5.3 * SIMMUL,
                    in1=bp,
                    op0=ALU.mult,
                    op1=ALU.subtract,
                )
                eb = bsmall.tile([128, W], F32, tag="eb")
                pcol = stats.tile([128, 1], F32, tag="pcol")
                nc.scalar.activation(
                    eb, ub, AF.Exp, scale=30.0 / SIMMUL, bias=bias150, accum_out=pcol
                )

                # combine: loss = softplus(ln(P) + ln(N))
                ncol = stats.tile([128, 1], F32, tag="ncol")
                nc.vector.reduce_sum(ncol, nsum, axis=mybir.AxisListType.X)
                lp = stats.tile([128, 1], F32, tag="lp")
                nc.scalar.activation(lp, pcol, AF.Ln)
                lnn = stats.tile([128, 1], F32, tag="lnn")
                nc.scalar.activation(lnn, ncol, AF.Ln)
                xr = stats.tile([128, 1], F32, tag="xr")
                nc.vector.tensor_tensor(xr, lp, lnn, op=ALU.add)
                er = stats.tile([128, 1], F32, tag="er")
                nc.scalar.activation(er, xr, AF.Exp)
                er1 = stats.tile([128, 1], F32, tag="er1")
                nc.vector.tensor_scalar_add(er1, er, 1.0)
                nc.scalar.activation(losses[:, m : m + 1], er1, AF.Ln)

            lsum = singles.tile([128, 1], F32)
            nc.vector.reduce_sum(lsum, losses, axis=mybir.AxisListType.X)
            nc.sync.dma_start(out=o_loss, in_=lsum)

    nc.compile()
    return nc


def _prep_inputs(feat: np.ndarray, label: np.ndarray):
    """Sort rows by label, cast to bf16, build per-core padded band masks."""
    perm = np.argsort(label, kind="stable")
    lab64 = np.asarray(label)[perm].astype(np.int64)
    feat_s = np.ascontiguousarray(
        np.asarray(feat, dtype=np.float32)[perm]
    ).astype(ml_dtypes.bfloat16)

    # verify every row's group fits its tile's band window
    starts = np.searchsorted(lab64, lab64, side="left")
    ends = np.searchsorted(lab64, lab64, side="right")
    rows = np.arange(N)
    woff = (rows // 128) * 128 - PAD  # window [woff, woff + W)
    assert (starts >= woff).all() and (ends <= woff + W).all(), (
        "label group exceeds band window; widen W"
    )

    in_maps = []
    for c in range(NCORES):
        sl = slice(c * R, (c + 1) * R)
        maskp = np.zeros((R, NP_), dtype=ml_dtypes.float8_e4m3fn)
        maskp[:, PAD : PAD + N] = (lab64[sl][:, None] == lab64[None, :]).astype(
            ml_dtypes.float8_e4m3fn
        )
        in_maps.append({"feat_in": feat_s, "mask_in": maskp})
    return in_maps


def kernel(feat: np.ndarray, label: np.ndarray) -> np.ndarray:
    feat = np.asarray(feat, dtype=np.float32)
    label = np.asarray(label)
    assert feat.shape == (N, D) and label.shape == (N,)

    in_maps = _prep_inputs(feat, label)

    if "nc" not in _CACHED:
        _CACHED["nc"] = _build_nc()
    nc = _CACHED["nc"]

    res = run_bass_kernel_spmd(nc, in_maps, core_ids=list(range(NCORES)))
    total = sum(float(res.results[c]["o_loss"].sum()) for c in range(NCORES))
    return np.float32(total / N)
